# revision 23
# baseline (speedup 1.0000x reference)
"""LSTM-pool kernel for Trainium2, 8-core data-parallel SPMD.

Math (per batch row b):
  x_t = [seq[b,t], seq_e[b,t], seq_t[b,t]]              (A = 384)
  z_t = x_t @ Wi + h_{t-1} @ Wh + bh                    (4F = 512, gates i,f,g,o)
  c_t = sig(f)*c_{t-1} + sig(i)*tanh(g);  h_t = sig(o)*tanh(c_t)
  out = relu([h_T, src] @ W1 + b1) @ W2 + b2

Implementation notes:
  * Host pre-transposes the 3 big [B,T,F] tensors to feature-major fp8e4m3
    (x/4, Wi*4 so PSUM holds the exact product) - no on-device casts or
    transposes, 4x less HBM traffic.
  * All z matmuls are fp8 DoubleRow (2 k-tiles/pass); the recurrent matmul
    keeps h in fp8 with a zero second k-tile so every PSUM accumulation
    group is mode-pure (mixing DR and plain matmuls in a group is fatal).
  * PSUM: one bank per gate quad, [128, 4, 512] tiles, double buffered
    (8 banks total).  Input projections run at N=512; per-chunk recurrent
    matmuls accumulate into 128-wide sub-regions.  Exactly one start=True
    per bank per step (start marks the whole 2KB bank pending-zero).
  * The g-quad columns of Wi/Wh are pre-scaled by 2 on the host so a
    single Sigmoid covers all four gates; tanh(g) = 2*sig(2g)-1 is
    reconstructed on the DVE (one tensor_scalar op).  This keeps ACT - the
    bottleneck engine - at two instructions per chunk-step.
  * Batch 512/core is pipelined as K=4 chunks of 128; each chunk's
    cell-update tail (tanh(c), h-mul) is emitted one chunk-slot late so
    the in-order ACT queue never head-of-line blocks on its own chunk.
  * Gate/cell arithmetic is bf16 on the DVE (4x mode); h is written
    directly in fp8 for the next recurrent matmul.
"""

import sys

sys.path.insert(0, "/opt/trn_rl_repo")

import numpy as np

import concourse.bass as bass
import concourse.mybir as mybir
import concourse.tile as tile
from concourse import bacc
from concourse.bass_utils import run_bass_kernel_spmd

dt = mybir.dt
AF = mybir.ActivationFunctionType
ALU = mybir.AluOpType
F8 = dt.np(dt.float8e4)
BF16 = dt.np(dt.bfloat16)

NCORES = 8
BFULL = 4096
B = BFULL // NCORES  # 512 batch rows per core
T = 128
F = 128
K = 4  # batch chunks per core
NH = B // K  # 128 rows per chunk
TC = 32  # time steps per DMA chunk
XSCALE = 4.0  # x shipped as x/XSCALE in fp8, Wi as Wi*XSCALE

# PSUM quad-bank order [g, i, f, o]; quad q -> Wi/Wh column block index
QUAD_COLS = [2, 0, 1, 3]


def build_nc(zero_bias: bool, t_steps: int = T):
    nc = bacc.Bacc("TRN2", target_bir_lowering=False, debug=False, num_devices=NCORES)

    xT = nc.dram_tensor("xT", [3, 128, T, B], dt.float8e4, kind="ExternalInput")
    wiP = nc.dram_tensor("wiP", [4, 2, 128, 2, 128], dt.float8e4, kind="ExternalInput")
    whP = nc.dram_tensor("whP", [4, 128, 2, 128], dt.float8e4, kind="ExternalInput")
    bh4 = nc.dram_tensor("bh4", [128, 4], dt.float32, kind="ExternalInput")
    srcT = nc.dram_tensor("srcT", [128, B], dt.bfloat16, kind="ExternalInput")
    w1b = nc.dram_tensor("w1b", [2, 128, 128], dt.bfloat16, kind="ExternalInput")
    w2b = nc.dram_tensor("w2b", [128, 128], dt.bfloat16, kind="ExternalInput")
    b1 = nc.dram_tensor("b1", [128], dt.float32, kind="ExternalInput")
    b2 = nc.dram_tensor("b2", [128], dt.float32, kind="ExternalInput")
    outT = nc.dram_tensor("outT", [F, B], dt.float32, kind="ExternalOutput")

    nchunk = (t_steps + TC - 1) // TC
    DR = mybir.MatmulPerfMode.DoubleRow

    with tile.TileContext(nc) as tc:
        with (
            tc.tile_pool(name="const", bufs=1) as constp,
            tc.tile_pool(name="gates", bufs=3) as gatep,
        ):
            # ---------------- weights / constants ----------------
            wi = constp.tile([128, 4, 2, 2, 128], dt.float8e4)
            nc.sync.dma_start(wi[:], wiP[:].rearrange("q pr k two m -> k q pr two m"))
            wh = constp.tile([128, 4, 2, 128], dt.float8e4)
            nc.sync.dma_start(wh[:], whP[:].rearrange("q k two m -> k q two m"))
            srcb = constp.tile([128, B], dt.bfloat16)
            nc.sync.dma_start(srcb[:], srcT[:])
            w1 = constp.tile([128, 2, 128], dt.bfloat16)
            nc.sync.dma_start(w1[:], w1b[:].rearrange("two k m -> k two m"))
            w2 = constp.tile([128, 128], dt.bfloat16)
            nc.sync.dma_start(w2[:], w2b[:])
            b1t = constp.tile([128, 1], dt.float32)
            nc.sync.dma_start(b1t[:], b1[:].rearrange("(f one) -> f one", one=1))
            b2t = constp.tile([128, 1], dt.float32)
            nc.sync.dma_start(b2t[:], b2[:].rearrange("(f one) -> f one", one=1))
            bias_g = constp.tile([128, 4], dt.float32)
            nc.sync.dma_start(bias_g[:], bh4[:])

            # ---------------- x staging (double buffer) ----------------
            # plane 3 of the kc dim is the DoubleRow zero pad - memset once.
            xts = []
            for i in range(2):
                xt = constp.tile([128, 4, TC, B], dt.float8e4, name=f"xt{i}")
                nc.gpsimd.memset(xt[:, 3, :, :], 0.0)
                xts.append(xt)

            def dma_chunk(ch):
                t0 = ch * TC
                nc.sync.dma_start(
                    xts[ch % 2][:, 0:3, :, :],
                    xT[:].rearrange("kc p t b -> p kc t b")[:, :, t0 : t0 + TC, :],
                )

            # ---------------- persistent state ----------------
            cs, hs = [], []
            for c in range(K):
                c_t = constp.tile([128, NH], dt.bfloat16, name=f"c_{c}")
                nc.gpsimd.memset(c_t[:], 0.0)
                cs.append(c_t)
                h_t = constp.tile([128, 2, NH], dt.float8e4, name=f"h_{c}")
                nc.gpsimd.memset(h_t[:], 0.0)
                hs.append(h_t)

            dma_chunk(0)

            zp_ctx = tc.tile_pool(name="zp", bufs=2, space="PSUM")
            zp = zp_ctx.__enter__()

            def emit_ip_quad(zt, t, q, with_stop):
                """input projection for step t, quad q (full batch, N=512).

                start_tensor_calc marks the whole 2KB bank pending-zero, so
                only the first matmul touching quad-bank q sets it.
                """
                buf = xts[(t // TC) % 2]
                ts_ = t % TC
                nc.tensor.matmul(
                    zt[:, q, :],
                    wi[:, q, 0, :, :],
                    buf[:, 0:2, ts_, :],
                    start=True,
                    stop=False,
                    perf_mode=DR,
                    skip_group_check=True,
                )
                nc.tensor.matmul(
                    zt[:, q, :],
                    wi[:, q, 1, :, :],
                    buf[:, 2:4, ts_, :],
                    start=False,
                    stop=with_stop,
                    perf_mode=DR,
                    skip_group_check=True,
                )

            def emit_rec(zt, c):
                bs = slice(c * NH, (c + 1) * NH)
                for q in range(4):
                    nc.tensor.matmul(
                        zt[:, q, bs],
                        wh[:, q, :, :],
                        hs[c][:],
                        start=False,
                        stop=(c == K - 1),
                        perf_mode=DR,
                        skip_group_check=True,
                    )

            def emit_gates(zt, t, c):
                """single sigmoid over all 4 quads (g pre-scaled by 2)."""
                bs = slice(c * NH, (c + 1) * NH)
                sg = gatep.tile(
                    [128, 4, NH], dt.bfloat16, tag=f"sg{c}", name=f"sg{c}_{t}"
                )
                if zero_bias:
                    nc.scalar.activation(sg[:], zt[:, :, bs], AF.Sigmoid)
                else:
                    for q in range(4):
                        nc.scalar.activation(
                            sg[:, q, :],
                            zt[:, q, bs],
                            AF.Sigmoid,
                            bias=bias_g[:, q : q + 1],
                        )
                return sg

            def emit_cell(t, c, sg):
                # tanh(g) = 2*sig(2g) - 1 (the 2x is pre-folded into the
                # g-quad weights).  Plain tensor_tensor ops run in the DVE
                # 4x perf mode; tensor_scalar does not, but is one op.
                tg = gatep.tile([128, NH], dt.bfloat16, tag=f"tg{c}", name=f"tg{c}_{t}")
                nc.vector.tensor_scalar(tg[:], sg[:, 0, :], 2.0, -1.0, ALU.mult, ALU.add)
                m2 = gatep.tile([128, NH], dt.bfloat16, tag=f"m2{c}", name=f"m2{c}_{t}")
                nc.vector.tensor_mul(m2[:], sg[:, 1, :], tg[:])
                m1 = gatep.tile([128, NH], dt.bfloat16, tag=f"m1{c}", name=f"m1{c}_{t}")
                nc.vector.tensor_mul(m1[:], sg[:, 2, :], cs[c][:])
                nc.vector.tensor_add(cs[c][:], m1[:], m2[:])

            def emit_tail(t, c, sg):
                tc2 = gatep.tile(
                    [128, NH], dt.bfloat16, tag=f"tc{c}", name=f"tc{c}_{t}"
                )
                nc.scalar.activation(tc2[:], cs[c][:], AF.Tanh)
                nc.vector.tensor_mul(hs[c][:, 0, :], sg[:, 3, :], tc2[:])

            # prologue: projections for t=0
            z_cur = zp.tile([128, 4, B], dt.float32, tag="z", name="z_p")
            for q in range(4):
                emit_ip_quad(z_cur, 0, q, with_stop=True)

            pending = None  # (t, c, sg) awaiting tail emission
            z_next = None
            for t in range(t_steps):
                ch = t // TC
                if t % TC == 0 and ch + 1 < nchunk:
                    dma_chunk(ch + 1)
                for c in range(K):
                    if t > 0:
                        emit_rec(z_cur, c)
                    # input projections for t+1 are emitted in slot 0, right
                    # after rec(c0), so the later slots' recurrent matmuls
                    # and next step's rec(c0) aren't queued behind them on
                    # the in-order PE.
                    if t + 1 < t_steps and c == 0:
                        z_next = zp.tile(
                            [128, 4, B], dt.float32, tag="z", name=f"z_{t + 1}"
                        )
                        for q in range(4):
                            emit_ip_quad(z_next, t + 1, q, with_stop=False)
                    sg = emit_gates(z_cur, t, c)
                    emit_cell(t, c, sg)
                    if pending is not None:
                        emit_tail(*pending)
                    pending = (t, c, sg)
                if t + 1 < t_steps:
                    z_cur = z_next
            emit_tail(*pending)

            zp_ctx.__exit__(None, None, None)

            # ---------------- merge layer ----------------
            hbf = constp.tile([128, K, NH], dt.bfloat16)
            for c in range(K):
                nc.scalar.activation(hbf[:, c, :], hs[c][:, 0, :], AF.Copy)
            with tc.tile_pool(name="mp", bufs=1, space="PSUM") as mp:
                ps_hid = mp.tile([128, B], dt.float32)
                for c in range(K):
                    bs = slice(c * NH, (c + 1) * NH)
                    nc.tensor.matmul(
                        ps_hid[:, bs], w1[:, 0, :], hbf[:, c, :], start=True, stop=False
                    )
                    nc.tensor.matmul(
                        ps_hid[:, bs], w1[:, 1, :], srcb[:, bs], start=False, stop=True
                    )
                hid_bf = constp.tile([128, B], dt.bfloat16)
                nc.scalar.activation(hid_bf[:], ps_hid[:], AF.Relu, bias=b1t[:])

                ps_out = mp.tile([128, B], dt.float32)
                nc.tensor.matmul(ps_out[:], w2[:], hid_bf[:], start=True, stop=True)
                out_sb = constp.tile([128, B], dt.float32)
                nc.scalar.activation(out_sb[:], ps_out[:], AF.Identity, bias=b2t[:])
                nc.sync.dma_start(outT[:], out_sb[:])

    nc.compile()
    return nc


_NC_CACHE: dict = {}


def _get_nc(zero_bias: bool):
    if zero_bias not in _NC_CACHE:
        _NC_CACHE[zero_bias] = build_nc(zero_bias)
    return _NC_CACHE[zero_bias]


def make_in_maps(**inputs):
    """Host-side reshaping: slice per core, pre-transpose, pre-quantize."""
    f32 = lambda x: np.asarray(x, dtype=np.float32)
    Wi = f32(inputs["Wi"])  # [384, 512]
    Wh = f32(inputs["Wh"])  # [128, 512]
    bh = f32(inputs["bh"])  # [512]
    W1 = f32(inputs["W1"])  # [256, 128]
    W2 = f32(inputs["W2"])  # [128, 128]
    b1 = f32(inputs["b1"])
    b2 = f32(inputs["b2"])

    # Wi packed for DoubleRow: [q, pair, k, two, m], scaled by XSCALE.
    # Wh packed for DoubleRow with a zero second k-tile: [q, k, two, m].
    # The g quad (and its bias) is additionally scaled by 2 so that
    # tanh(g) = 2*sigmoid(2g) - 1 comes out of the shared sigmoid.
    wiP = np.zeros((4, 2, 128, 2, 128), np.float32)
    whP = np.zeros((4, 128, 2, 128), np.float32)
    bh4 = np.zeros((128, 4), np.float32)
    for q, blk in enumerate(QUAD_COLS):
        gs = 2.0 if q == 0 else 1.0
        colsl = slice(blk * 128, (blk + 1) * 128)
        for kc in range(3):
            wiP[q, kc // 2, :, kc % 2, :] = (
                gs * XSCALE * Wi[kc * 128 : (kc + 1) * 128, colsl]
            )
        whP[q, :, 0, :] = gs * Wh[:, colsl]
        bh4[:, q] = gs * bh[colsl]
    wiP = wiP.astype(F8)
    whP = whP.astype(F8)
    w1b = np.stack([W1[0:128, :], W1[128:256, :]]).astype(BF16)
    w2b = W2.astype(BF16)

    shared = {
        "wiP": wiP,
        "whP": whP,
        "bh4": np.ascontiguousarray(bh4),
        "w1b": w1b,
        "w2b": w2b,
        "b1": b1,
        "b2": b2,
    }

    # big tensors: cast full arrays to fp8 once, then per-core transpose
    planes = []
    for nm in ("seq", "seq_e", "seq_t"):
        a = np.asarray(inputs[nm])
        planes.append((a * (1.0 / XSCALE)).astype(F8))  # [4096, T, F]
    src = f32(inputs["src"])

    in_maps = []
    for c in range(NCORES):
        sl = slice(c * B, (c + 1) * B)
        m = dict(shared)
        xT = np.empty((3, 128, T, B), F8)
        for kc in range(3):
            xT[kc] = planes[kc][sl].transpose(2, 1, 0)
        m["xT"] = xT
        m["srcT"] = np.ascontiguousarray(src[sl].T).astype(BF16)
        in_maps.append(m)
    return in_maps


def kernel(**inputs) -> np.ndarray:
    zero_bias = not np.any(np.asarray(inputs["bh"]))
    nc = _get_nc(zero_bias)
    in_maps = make_in_maps(**inputs)
    res = run_bass_kernel_spmd(nc, in_maps, core_ids=list(range(NCORES)))
    out = np.empty((BFULL, F), np.float32)
    for c in range(NCORES):
        out[c * B : (c + 1) * B] = res.results[c]["outT"].T
    return out


# revision 25
# speedup vs baseline: 1.0126x; 1.0126x over previous
"""LSTM-pool kernel for Trainium2, 8-core data-parallel SPMD.

Math (per batch row b):
  x_t = [seq[b,t], seq_e[b,t], seq_t[b,t]]              (A = 384)
  z_t = x_t @ Wi + h_{t-1} @ Wh + bh                    (4F = 512, gates i,f,g,o)
  c_t = sig(f)*c_{t-1} + sig(i)*tanh(g);  h_t = sig(o)*tanh(c_t)
  out = relu([h_T, src] @ W1 + b1) @ W2 + b2

Implementation notes:
  * Host pre-transposes the 3 big [B,T,F] tensors to feature-major fp8e4m3
    (x/4, Wi*4 so PSUM holds the exact product) - no on-device casts or
    transposes, 4x less HBM traffic.
  * All z matmuls are fp8 DoubleRow (2 k-tiles/pass); the recurrent matmul
    keeps h in fp8 with a zero second k-tile so every PSUM accumulation
    group is mode-pure (mixing DR and plain matmuls in a group is fatal).
  * PSUM: one bank per gate quad, [128, 4, 512] tiles, double buffered
    (8 banks total).  Input projections run at N=512; per-chunk recurrent
    matmuls accumulate into 128-wide sub-regions.  Exactly one start=True
    per bank per step (start marks the whole 2KB bank pending-zero).
  * The g-quad columns of Wi/Wh are pre-scaled by 2 on the host so a
    single Sigmoid covers all four gates; tanh(g) = 2*sig(2g)-1 is
    reconstructed on the DVE (one tensor_scalar op).  This keeps ACT - the
    bottleneck engine - at two instructions per chunk-step.
  * Batch 512/core is pipelined as K=4 chunks of 128; each chunk's
    cell-update tail (tanh(c), h-mul) is emitted one chunk-slot late so
    the in-order ACT queue never head-of-line blocks on its own chunk.
  * Gate/cell arithmetic is bf16 on the DVE (4x mode); h is written
    directly in fp8 for the next recurrent matmul.
"""

import sys

sys.path.insert(0, "/opt/trn_rl_repo")

import numpy as np

import concourse.bass as bass
import concourse.mybir as mybir
import concourse.tile as tile
from concourse import bacc
from concourse.bass_utils import run_bass_kernel_spmd

dt = mybir.dt
AF = mybir.ActivationFunctionType
ALU = mybir.AluOpType
F8 = dt.np(dt.float8e4)
BF16 = dt.np(dt.bfloat16)

NCORES = 8
BFULL = 4096
B = BFULL // NCORES  # 512 batch rows per core
T = 128
F = 128
K = 4  # batch chunks per core
NH = B // K  # 128 rows per chunk
TC = 16  # time steps per DMA chunk
XSCALE = 4.0  # x shipped as x/XSCALE in fp8, Wi as Wi*XSCALE

# PSUM quad-bank order [g, i, f, o]; quad q -> Wi/Wh column block index
QUAD_COLS = [2, 0, 1, 3]


def build_nc(zero_bias: bool, t_steps: int = T):
    nc = bacc.Bacc("TRN2", target_bir_lowering=False, debug=False, num_devices=NCORES)

    xT = nc.dram_tensor("xT", [3, 128, T, B], dt.float8e4, kind="ExternalInput")
    wiP = nc.dram_tensor("wiP", [4, 2, 128, 2, 128], dt.float8e4, kind="ExternalInput")
    whP = nc.dram_tensor("whP", [4, 128, 2, 128], dt.float8e4, kind="ExternalInput")
    bh4 = nc.dram_tensor("bh4", [128, 4], dt.float32, kind="ExternalInput")
    srcT = nc.dram_tensor("srcT", [128, B], dt.bfloat16, kind="ExternalInput")
    w1b = nc.dram_tensor("w1b", [2, 128, 128], dt.bfloat16, kind="ExternalInput")
    w2b = nc.dram_tensor("w2b", [128, 128], dt.bfloat16, kind="ExternalInput")
    b1 = nc.dram_tensor("b1", [128], dt.float32, kind="ExternalInput")
    b2 = nc.dram_tensor("b2", [128], dt.float32, kind="ExternalInput")
    outT = nc.dram_tensor("outT", [F, B], dt.float32, kind="ExternalOutput")

    nchunk = (t_steps + TC - 1) // TC
    DR = mybir.MatmulPerfMode.DoubleRow

    with tile.TileContext(nc) as tc:
        with (
            tc.tile_pool(name="const", bufs=1) as constp,
            tc.tile_pool(name="gates", bufs=3) as gatep,
        ):
            # ---------------- weights / constants ----------------
            wi = constp.tile([128, 4, 2, 2, 128], dt.float8e4)
            nc.sync.dma_start(wi[:], wiP[:].rearrange("q pr k two m -> k q pr two m"))
            wh = constp.tile([128, 4, 2, 128], dt.float8e4)
            nc.sync.dma_start(wh[:], whP[:].rearrange("q k two m -> k q two m"))
            srcb = constp.tile([128, B], dt.bfloat16)
            nc.sync.dma_start(srcb[:], srcT[:])
            w1 = constp.tile([128, 2, 128], dt.bfloat16)
            nc.sync.dma_start(w1[:], w1b[:].rearrange("two k m -> k two m"))
            w2 = constp.tile([128, 128], dt.bfloat16)
            nc.sync.dma_start(w2[:], w2b[:])
            b1t = constp.tile([128, 1], dt.float32)
            nc.sync.dma_start(b1t[:], b1[:].rearrange("(f one) -> f one", one=1))
            b2t = constp.tile([128, 1], dt.float32)
            nc.sync.dma_start(b2t[:], b2[:].rearrange("(f one) -> f one", one=1))
            bias_g = constp.tile([128, 4], dt.float32)
            nc.sync.dma_start(bias_g[:], bh4[:])

            # ---------------- x staging (double buffer) ----------------
            # plane 3 of the kc dim is the DoubleRow zero pad - memset once.
            xts = []
            for i in range(2):
                xt = constp.tile([128, 4, TC, B], dt.float8e4, name=f"xt{i}")
                nc.gpsimd.memset(xt[:, 3, :, :], 0.0)
                xts.append(xt)

            def dma_chunk(ch):
                t0 = ch * TC
                nc.sync.dma_start(
                    xts[ch % 2][:, 0:3, :, :],
                    xT[:].rearrange("kc p t b -> p kc t b")[:, :, t0 : t0 + TC, :],
                )

            # ---------------- persistent state ----------------
            cs, hs = [], []
            for c in range(K):
                c_t = constp.tile([128, NH], dt.bfloat16, name=f"c_{c}")
                nc.gpsimd.memset(c_t[:], 0.0)
                cs.append(c_t)
                h_t = constp.tile([128, 2, NH], dt.float8e4, name=f"h_{c}")
                nc.gpsimd.memset(h_t[:], 0.0)
                hs.append(h_t)

            dma_chunk(0)

            zp_ctx = tc.tile_pool(name="zp", bufs=2, space="PSUM")
            zp = zp_ctx.__enter__()

            def emit_ip_quad(zt, t, q, with_stop):
                """input projection for step t, quad q (full batch, N=512).

                start_tensor_calc marks the whole 2KB bank pending-zero, so
                only the first matmul touching quad-bank q sets it.
                """
                buf = xts[(t // TC) % 2]
                ts_ = t % TC
                nc.tensor.matmul(
                    zt[:, q, :],
                    wi[:, q, 0, :, :],
                    buf[:, 0:2, ts_, :],
                    start=True,
                    stop=False,
                    perf_mode=DR,
                    skip_group_check=True,
                )
                nc.tensor.matmul(
                    zt[:, q, :],
                    wi[:, q, 1, :, :],
                    buf[:, 2:4, ts_, :],
                    start=False,
                    stop=with_stop,
                    perf_mode=DR,
                    skip_group_check=True,
                )

            def emit_rec(zt, c):
                bs = slice(c * NH, (c + 1) * NH)
                for q in range(4):
                    nc.tensor.matmul(
                        zt[:, q, bs],
                        wh[:, q, :, :],
                        hs[c][:],
                        start=False,
                        stop=(c == K - 1),
                        perf_mode=DR,
                        skip_group_check=True,
                    )

            def emit_gates(zt, t, c):
                """single sigmoid over all 4 quads (g pre-scaled by 2)."""
                bs = slice(c * NH, (c + 1) * NH)
                sg = gatep.tile(
                    [128, 4, NH], dt.bfloat16, tag=f"sg{c}", name=f"sg{c}_{t}"
                )
                if zero_bias:
                    nc.scalar.activation(sg[:], zt[:, :, bs], AF.Sigmoid)
                else:
                    for q in range(4):
                        nc.scalar.activation(
                            sg[:, q, :],
                            zt[:, q, bs],
                            AF.Sigmoid,
                            bias=bias_g[:, q : q + 1],
                        )
                return sg

            def emit_cell(t, c, sg):
                # tanh(g) = 2*sig(2g) - 1 (the 2x is pre-folded into the
                # g-quad weights).  Plain tensor_tensor ops run in the DVE
                # 4x perf mode; tensor_scalar does not, but is one op.
                tg = gatep.tile([128, NH], dt.bfloat16, tag=f"tg{c}", name=f"tg{c}_{t}")
                nc.vector.tensor_scalar(tg[:], sg[:, 0, :], 2.0, -1.0, ALU.mult, ALU.add)
                m2 = gatep.tile([128, NH], dt.bfloat16, tag=f"m2{c}", name=f"m2{c}_{t}")
                nc.vector.tensor_mul(m2[:], sg[:, 1, :], tg[:])
                m1 = gatep.tile([128, NH], dt.bfloat16, tag=f"m1{c}", name=f"m1{c}_{t}")
                nc.vector.tensor_mul(m1[:], sg[:, 2, :], cs[c][:])
                nc.vector.tensor_add(cs[c][:], m1[:], m2[:])

            def emit_tail(t, c, sg):
                tc2 = gatep.tile(
                    [128, NH], dt.bfloat16, tag=f"tc{c}", name=f"tc{c}_{t}"
                )
                nc.scalar.activation(tc2[:], cs[c][:], AF.Tanh)
                # h-mul runs on the (otherwise idle) gpsimd engine so it is
                # not queued behind the next chunk's cell ops on the DVE
                nc.gpsimd.tensor_mul(hs[c][:, 0, :], sg[:, 3, :], tc2[:])

            # prologue: projections for t=0
            z_cur = zp.tile([128, 4, B], dt.float32, tag="z", name="z_p")
            for q in range(4):
                emit_ip_quad(z_cur, 0, q, with_stop=True)

            pending = None  # (t, c, sg) awaiting tail emission
            z_next = None
            for t in range(t_steps):
                ch = t // TC
                if t % TC == 0 and ch + 1 < nchunk:
                    dma_chunk(ch + 1)
                for c in range(K):
                    if t > 0:
                        emit_rec(z_cur, c)
                    # input projections for t+1 are emitted in slot 0, right
                    # after rec(c0), so the later slots' recurrent matmuls
                    # and next step's rec(c0) aren't queued behind them on
                    # the in-order PE.
                    if t + 1 < t_steps and c == 0:
                        z_next = zp.tile(
                            [128, 4, B], dt.float32, tag="z", name=f"z_{t + 1}"
                        )
                        for q in range(4):
                            emit_ip_quad(z_next, t + 1, q, with_stop=False)
                    sg = emit_gates(z_cur, t, c)
                    emit_cell(t, c, sg)
                    if pending is not None:
                        emit_tail(*pending)
                    pending = (t, c, sg)
                if t + 1 < t_steps:
                    z_cur = z_next
            emit_tail(*pending)

            zp_ctx.__exit__(None, None, None)

            # ---------------- merge layer ----------------
            hbf = constp.tile([128, K, NH], dt.bfloat16)
            for c in range(K):
                nc.scalar.activation(hbf[:, c, :], hs[c][:, 0, :], AF.Copy)
            with tc.tile_pool(name="mp", bufs=1, space="PSUM") as mp:
                ps_hid = mp.tile([128, B], dt.float32)
                for c in range(K):
                    bs = slice(c * NH, (c + 1) * NH)
                    nc.tensor.matmul(
                        ps_hid[:, bs], w1[:, 0, :], hbf[:, c, :], start=True, stop=False
                    )
                    nc.tensor.matmul(
                        ps_hid[:, bs], w1[:, 1, :], srcb[:, bs], start=False, stop=True
                    )
                hid_bf = constp.tile([128, B], dt.bfloat16)
                nc.scalar.activation(hid_bf[:], ps_hid[:], AF.Relu, bias=b1t[:])

                ps_out = mp.tile([128, B], dt.float32)
                nc.tensor.matmul(ps_out[:], w2[:], hid_bf[:], start=True, stop=True)
                out_sb = constp.tile([128, B], dt.float32)
                nc.scalar.activation(out_sb[:], ps_out[:], AF.Identity, bias=b2t[:])
                nc.sync.dma_start(outT[:], out_sb[:])

    nc.compile()
    return nc


_NC_CACHE: dict = {}


def _get_nc(zero_bias: bool):
    if zero_bias not in _NC_CACHE:
        _NC_CACHE[zero_bias] = build_nc(zero_bias)
    return _NC_CACHE[zero_bias]


def make_in_maps(**inputs):
    """Host-side reshaping: slice per core, pre-transpose, pre-quantize."""
    f32 = lambda x: np.asarray(x, dtype=np.float32)
    Wi = f32(inputs["Wi"])  # [384, 512]
    Wh = f32(inputs["Wh"])  # [128, 512]
    bh = f32(inputs["bh"])  # [512]
    W1 = f32(inputs["W1"])  # [256, 128]
    W2 = f32(inputs["W2"])  # [128, 128]
    b1 = f32(inputs["b1"])
    b2 = f32(inputs["b2"])

    # Wi packed for DoubleRow: [q, pair, k, two, m], scaled by XSCALE.
    # Wh packed for DoubleRow with a zero second k-tile: [q, k, two, m].
    # The g quad (and its bias) is additionally scaled by 2 so that
    # tanh(g) = 2*sigmoid(2g) - 1 comes out of the shared sigmoid.
    wiP = np.zeros((4, 2, 128, 2, 128), np.float32)
    whP = np.zeros((4, 128, 2, 128), np.float32)
    bh4 = np.zeros((128, 4), np.float32)
    for q, blk in enumerate(QUAD_COLS):
        gs = 2.0 if q == 0 else 1.0
        colsl = slice(blk * 128, (blk + 1) * 128)
        for kc in range(3):
            wiP[q, kc // 2, :, kc % 2, :] = (
                gs * XSCALE * Wi[kc * 128 : (kc + 1) * 128, colsl]
            )
        whP[q, :, 0, :] = gs * Wh[:, colsl]
        bh4[:, q] = gs * bh[colsl]
    wiP = wiP.astype(F8)
    whP = whP.astype(F8)
    w1b = np.stack([W1[0:128, :], W1[128:256, :]]).astype(BF16)
    w2b = W2.astype(BF16)

    shared = {
        "wiP": wiP,
        "whP": whP,
        "bh4": np.ascontiguousarray(bh4),
        "w1b": w1b,
        "w2b": w2b,
        "b1": b1,
        "b2": b2,
    }

    # big tensors: cast full arrays to fp8 once, then per-core transpose
    planes = []
    for nm in ("seq", "seq_e", "seq_t"):
        a = np.asarray(inputs[nm])
        planes.append((a * (1.0 / XSCALE)).astype(F8))  # [4096, T, F]
    src = f32(inputs["src"])

    in_maps = []
    for c in range(NCORES):
        sl = slice(c * B, (c + 1) * B)
        m = dict(shared)
        xT = np.empty((3, 128, T, B), F8)
        for kc in range(3):
            xT[kc] = planes[kc][sl].transpose(2, 1, 0)
        m["xT"] = xT
        m["srcT"] = np.ascontiguousarray(src[sl].T).astype(BF16)
        in_maps.append(m)
    return in_maps


def kernel(**inputs) -> np.ndarray:
    zero_bias = not np.any(np.asarray(inputs["bh"]))
    nc = _get_nc(zero_bias)
    in_maps = make_in_maps(**inputs)
    res = run_bass_kernel_spmd(nc, in_maps, core_ids=list(range(NCORES)))
    out = np.empty((BFULL, F), np.float32)
    for c in range(NCORES):
        out[c * B : (c + 1) * B] = res.results[c]["outT"].T
    return out


# revision 26
# speedup vs baseline: 1.1067x; 1.0930x over previous
"""LSTM-pool kernel for Trainium2, 8-core data-parallel SPMD.

Math (per batch row b):
  x_t = [seq[b,t], seq_e[b,t], seq_t[b,t]]              (A = 384)
  z_t = x_t @ Wi + h_{t-1} @ Wh + bh                    (4F = 512, gates i,f,g,o)
  c_t = sig(f)*c_{t-1} + sig(i)*tanh(g);  h_t = sig(o)*tanh(c_t)
  out = relu([h_T, src] @ W1 + b1) @ W2 + b2

Implementation notes:
  * Host pre-transposes the 3 big [B,T,F] tensors to feature-major fp8e4m3
    (x/4, Wi*4 so PSUM holds the exact product) - no on-device casts or
    transposes, 4x less HBM traffic.
  * All z matmuls are fp8 DoubleRow (2 k-tiles/pass); the recurrent matmul
    keeps h in fp8 with a zero second k-tile so every PSUM accumulation
    group is mode-pure (mixing DR and plain matmuls in a group is fatal).
  * PSUM: one bank per gate quad, [128, 4, 512] tiles, double buffered
    (8 banks total).  Input projections run at N=512; per-chunk recurrent
    matmuls accumulate into 128-wide sub-regions.  Exactly one start=True
    per bank per step (start marks the whole 2KB bank pending-zero).
  * The g-quad columns of Wi/Wh are pre-scaled by 2 on the host so a
    single Sigmoid covers all four gates; tanh(g) = 2*sig(2g)-1 is
    reconstructed on the DVE (one tensor_scalar op).  This keeps ACT - the
    bottleneck engine - at two instructions per chunk-step.
  * Batch 512/core is pipelined as K=4 chunks of 128; each chunk's
    cell-update tail (tanh(c), h-mul) is emitted one chunk-slot late so
    the in-order ACT queue never head-of-line blocks on its own chunk.
  * Gate/cell arithmetic is bf16 on the DVE (4x mode); h is written
    directly in fp8 for the next recurrent matmul.
"""

import sys

sys.path.insert(0, "/opt/trn_rl_repo")

import numpy as np

import concourse.bass as bass
import concourse.mybir as mybir
import concourse.tile as tile
from concourse import bacc
from concourse.bass_utils import run_bass_kernel_spmd

dt = mybir.dt
AF = mybir.ActivationFunctionType
ALU = mybir.AluOpType
F8 = dt.np(dt.float8e4)
BF16 = dt.np(dt.bfloat16)

NCORES = 8
BFULL = 4096
B = BFULL // NCORES  # 512 batch rows per core
T = 128
F = 128
K = 4  # batch chunks per core
NH = B // K  # 128 rows per chunk
TC = 16  # time steps per DMA chunk
XSCALE = 4.0  # x shipped as x/XSCALE in fp8, Wi as Wi*XSCALE

# PSUM quad-bank order [g, i, f, o]; quad q -> Wi/Wh column block index
QUAD_COLS = [2, 0, 1, 3]


def build_nc(zero_bias: bool, t_steps: int = T):
    nc = bacc.Bacc("TRN2", target_bir_lowering=False, debug=False, num_devices=NCORES)

    xT = nc.dram_tensor("xT", [3, 128, T, B], dt.float8e4, kind="ExternalInput")
    wiP = nc.dram_tensor("wiP", [4, 2, 128, 2, 128], dt.float8e4, kind="ExternalInput")
    whP = nc.dram_tensor("whP", [4, 128, 2, 128], dt.float8e4, kind="ExternalInput")
    bh4 = nc.dram_tensor("bh4", [128, 4], dt.float32, kind="ExternalInput")
    srcT = nc.dram_tensor("srcT", [128, B], dt.bfloat16, kind="ExternalInput")
    w1b = nc.dram_tensor("w1b", [2, 128, 128], dt.bfloat16, kind="ExternalInput")
    w2b = nc.dram_tensor("w2b", [128, 128], dt.bfloat16, kind="ExternalInput")
    b1 = nc.dram_tensor("b1", [128], dt.float32, kind="ExternalInput")
    b2 = nc.dram_tensor("b2", [128], dt.float32, kind="ExternalInput")
    outT = nc.dram_tensor("outT", [F, B], dt.float32, kind="ExternalOutput")

    nchunk = (t_steps + TC - 1) // TC
    DR = mybir.MatmulPerfMode.DoubleRow

    with tile.TileContext(nc) as tc:
        with (
            tc.tile_pool(name="const", bufs=1) as constp,
            tc.tile_pool(name="gates", bufs=3) as gatep,
        ):
            # ---------------- weights / constants ----------------
            wi = constp.tile([128, 4, 2, 2, 128], dt.float8e4)
            nc.sync.dma_start(wi[:], wiP[:].rearrange("q pr k two m -> k q pr two m"))
            wh = constp.tile([128, 4, 2, 128], dt.float8e4)
            nc.sync.dma_start(wh[:], whP[:].rearrange("q k two m -> k q two m"))
            srcb = constp.tile([128, B], dt.bfloat16)
            nc.sync.dma_start(srcb[:], srcT[:])
            w1 = constp.tile([128, 2, 128], dt.bfloat16)
            nc.sync.dma_start(w1[:], w1b[:].rearrange("two k m -> k two m"))
            w2 = constp.tile([128, 128], dt.bfloat16)
            nc.sync.dma_start(w2[:], w2b[:])
            b1t = constp.tile([128, 1], dt.float32)
            nc.sync.dma_start(b1t[:], b1[:].rearrange("(f one) -> f one", one=1))
            b2t = constp.tile([128, 1], dt.float32)
            nc.sync.dma_start(b2t[:], b2[:].rearrange("(f one) -> f one", one=1))
            bias_g = constp.tile([128, 4], dt.float32)
            nc.sync.dma_start(bias_g[:], bh4[:])

            # ---------------- x staging (double buffer) ----------------
            # plane 3 of the kc dim is the DoubleRow zero pad - memset once.
            xts = []
            for i in range(2):
                xt = constp.tile([128, 4, TC, B], dt.float8e4, name=f"xt{i}")
                nc.gpsimd.memset(xt[:, 3, :, :], 0.0)
                xts.append(xt)

            def dma_chunk(ch):
                t0 = ch * TC
                nc.sync.dma_start(
                    xts[ch % 2][:, 0:3, :, :],
                    xT[:].rearrange("kc p t b -> p kc t b")[:, :, t0 : t0 + TC, :],
                )

            # ---------------- persistent state ----------------
            cs, hs = [], []
            for c in range(K):
                c_t = constp.tile([128, NH], dt.bfloat16, name=f"c_{c}")
                nc.gpsimd.memset(c_t[:], 0.0)
                cs.append(c_t)
                h_t = constp.tile([128, 2, NH], dt.float8e4, name=f"h_{c}")
                nc.gpsimd.memset(h_t[:], 0.0)
                hs.append(h_t)

            dma_chunk(0)

            zp_ctx = tc.tile_pool(name="zp", bufs=2, space="PSUM")
            zp = zp_ctx.__enter__()

            def emit_ip(zt, t, c, with_stop):
                """input projection for step t, chunk c, into its PSUM bank."""
                buf = xts[(t // TC) % 2]
                ts_ = t % TC
                bs = slice(c * NH, (c + 1) * NH)
                for q in range(4):
                    nc.tensor.matmul(
                        zt[:, q, :],
                        wi[:, q, 0, :, :],
                        buf[:, 0:2, ts_, bs],
                        start=(q == 0),
                        stop=False,
                        perf_mode=DR,
                        skip_group_check=True,
                    )
                    nc.tensor.matmul(
                        zt[:, q, :],
                        wi[:, q, 1, :, :],
                        buf[:, 2:4, ts_, bs],
                        start=False,
                        stop=(with_stop and q == 3),
                        perf_mode=DR,
                        skip_group_check=True,
                    )

            def emit_rec(zt, c):
                for q in range(4):
                    nc.tensor.matmul(
                        zt[:, q, :],
                        wh[:, q, :, :],
                        hs[c][:],
                        start=False,
                        stop=(q == 3),
                        perf_mode=DR,
                        skip_group_check=True,
                    )

            def emit_gates(zt, t, c):
                """single sigmoid over all 4 quads (g pre-scaled by 2)."""
                sg = gatep.tile(
                    [128, 4, NH], dt.bfloat16, tag=f"sg{c}", name=f"sg{c}_{t}"
                )
                if zero_bias:
                    nc.scalar.activation(sg[:], zt[:], AF.Sigmoid)
                else:
                    for q in range(4):
                        nc.scalar.activation(
                            sg[:, q, :],
                            zt[:, q, :],
                            AF.Sigmoid,
                            bias=bias_g[:, q : q + 1],
                        )
                return sg

            def emit_cell(t, c, sg):
                # tanh(g) = 2*sig(2g) - 1 (the 2x is pre-folded into the
                # g-quad weights).  Plain tensor_tensor ops run in the DVE
                # 4x perf mode; tensor_scalar does not, but is one op.
                tg = gatep.tile([128, NH], dt.bfloat16, tag=f"tg{c}", name=f"tg{c}_{t}")
                nc.vector.tensor_scalar(tg[:], sg[:, 0, :], 2.0, -1.0, ALU.mult, ALU.add)
                m2 = gatep.tile([128, NH], dt.bfloat16, tag=f"m2{c}", name=f"m2{c}_{t}")
                nc.vector.tensor_mul(m2[:], sg[:, 1, :], tg[:])
                m1 = gatep.tile([128, NH], dt.bfloat16, tag=f"m1{c}", name=f"m1{c}_{t}")
                nc.vector.tensor_mul(m1[:], sg[:, 2, :], cs[c][:])
                nc.vector.tensor_add(cs[c][:], m1[:], m2[:])

            def emit_tail(t, c, sg):
                tc2 = gatep.tile(
                    [128, NH], dt.bfloat16, tag=f"tc{c}", name=f"tc{c}_{t}"
                )
                nc.scalar.activation(tc2[:], cs[c][:], AF.Tanh)
                # h-mul runs on the (otherwise idle) gpsimd engine so it is
                # not queued behind the next chunk's cell ops on the DVE
                nc.gpsimd.tensor_mul(hs[c][:, 0, :], sg[:, 3, :], tc2[:])

            # prologue: projections for t=0
            z_cur = []
            for c in range(K):
                zt = zp.tile([128, 4, NH], dt.float32, tag=f"z{c}", name=f"z{c}_p")
                emit_ip(zt, 0, c, with_stop=True)
                z_cur.append(zt)

            pending = None  # (t, c, sg) awaiting tail emission
            for t in range(t_steps):
                ch = t // TC
                if t % TC == 0 and ch + 1 < nchunk:
                    dma_chunk(ch + 1)
                for c in range(K):
                    zt = z_cur[c]
                    if t > 0:
                        emit_rec(zt, c)
                    if t + 1 < t_steps:
                        zn = zp.tile(
                            [128, 4, NH], dt.float32, tag=f"z{c}", name=f"z{c}_{t + 1}"
                        )
                        emit_ip(zn, t + 1, c, with_stop=False)
                        z_cur[c] = zn
                    sg = emit_gates(zt, t, c)
                    emit_cell(t, c, sg)
                    if pending is not None:
                        emit_tail(*pending)
                    pending = (t, c, sg)
            emit_tail(*pending)

            zp_ctx.__exit__(None, None, None)

            # ---------------- merge layer ----------------
            hbf = constp.tile([128, K, NH], dt.bfloat16)
            for c in range(K):
                nc.scalar.activation(hbf[:, c, :], hs[c][:, 0, :], AF.Copy)
            with tc.tile_pool(name="mp", bufs=1, space="PSUM") as mp:
                ps_hid = mp.tile([128, B], dt.float32)
                for c in range(K):
                    bs = slice(c * NH, (c + 1) * NH)
                    nc.tensor.matmul(
                        ps_hid[:, bs], w1[:, 0, :], hbf[:, c, :], start=True, stop=False
                    )
                    nc.tensor.matmul(
                        ps_hid[:, bs], w1[:, 1, :], srcb[:, bs], start=False, stop=True
                    )
                hid_bf = constp.tile([128, B], dt.bfloat16)
                nc.scalar.activation(hid_bf[:], ps_hid[:], AF.Relu, bias=b1t[:])

                ps_out = mp.tile([128, B], dt.float32)
                nc.tensor.matmul(ps_out[:], w2[:], hid_bf[:], start=True, stop=True)
                out_sb = constp.tile([128, B], dt.float32)
                nc.scalar.activation(out_sb[:], ps_out[:], AF.Identity, bias=b2t[:])
                nc.sync.dma_start(outT[:], out_sb[:])

    nc.compile()
    return nc


_NC_CACHE: dict = {}


def _get_nc(zero_bias: bool):
    if zero_bias not in _NC_CACHE:
        _NC_CACHE[zero_bias] = build_nc(zero_bias)
    return _NC_CACHE[zero_bias]


def make_in_maps(**inputs):
    """Host-side reshaping: slice per core, pre-transpose, pre-quantize."""
    f32 = lambda x: np.asarray(x, dtype=np.float32)
    Wi = f32(inputs["Wi"])  # [384, 512]
    Wh = f32(inputs["Wh"])  # [128, 512]
    bh = f32(inputs["bh"])  # [512]
    W1 = f32(inputs["W1"])  # [256, 128]
    W2 = f32(inputs["W2"])  # [128, 128]
    b1 = f32(inputs["b1"])
    b2 = f32(inputs["b2"])

    # Wi packed for DoubleRow: [q, pair, k, two, m], scaled by XSCALE.
    # Wh packed for DoubleRow with a zero second k-tile: [q, k, two, m].
    # The g quad (and its bias) is additionally scaled by 2 so that
    # tanh(g) = 2*sigmoid(2g) - 1 comes out of the shared sigmoid.
    wiP = np.zeros((4, 2, 128, 2, 128), np.float32)
    whP = np.zeros((4, 128, 2, 128), np.float32)
    bh4 = np.zeros((128, 4), np.float32)
    for q, blk in enumerate(QUAD_COLS):
        gs = 2.0 if q == 0 else 1.0
        colsl = slice(blk * 128, (blk + 1) * 128)
        for kc in range(3):
            wiP[q, kc // 2, :, kc % 2, :] = (
                gs * XSCALE * Wi[kc * 128 : (kc + 1) * 128, colsl]
            )
        whP[q, :, 0, :] = gs * Wh[:, colsl]
        bh4[:, q] = gs * bh[colsl]
    wiP = wiP.astype(F8)
    whP = whP.astype(F8)
    w1b = np.stack([W1[0:128, :], W1[128:256, :]]).astype(BF16)
    w2b = W2.astype(BF16)

    shared = {
        "wiP": wiP,
        "whP": whP,
        "bh4": np.ascontiguousarray(bh4),
        "w1b": w1b,
        "w2b": w2b,
        "b1": b1,
        "b2": b2,
    }

    # big tensors: cast full arrays to fp8 once, then per-core transpose
    planes = []
    for nm in ("seq", "seq_e", "seq_t"):
        a = np.asarray(inputs[nm])
        planes.append((a * (1.0 / XSCALE)).astype(F8))  # [4096, T, F]
    src = f32(inputs["src"])

    in_maps = []
    for c in range(NCORES):
        sl = slice(c * B, (c + 1) * B)
        m = dict(shared)
        xT = np.empty((3, 128, T, B), F8)
        for kc in range(3):
            xT[kc] = planes[kc][sl].transpose(2, 1, 0)
        m["xT"] = xT
        m["srcT"] = np.ascontiguousarray(src[sl].T).astype(BF16)
        in_maps.append(m)
    return in_maps


def kernel(**inputs) -> np.ndarray:
    zero_bias = not np.any(np.asarray(inputs["bh"]))
    nc = _get_nc(zero_bias)
    in_maps = make_in_maps(**inputs)
    res = run_bass_kernel_spmd(nc, in_maps, core_ids=list(range(NCORES)))
    out = np.empty((BFULL, F), np.float32)
    for c in range(NCORES):
        out[c * B : (c + 1) * B] = res.results[c]["outT"].T
    return out
